# revision 9
# baseline (speedup 1.0000x reference)
"""Trainium2 Bass kernel for nn_DeepRTE (sparse_attention).

Self-contained: hardcodes all shapes. Shards batch*collocation across 8
NeuronCores (each core: one batch b = core//2, 16 of 32 collocation
points n). All arithmetic happens on-device; the host only slices,
replicates/packs layouts, and casts dtypes.

Per-core pipeline:
  A. geometry (pos_local/ang_local/rel_dist) in NK layout [272 = n*17+k1, 128 p]
  B. attention MLP 6->64->64->1, 2-block-packed feature-major
     [12, 17408 cols=(kn,p)], masked softmax over p, coeff = exp(-attn@sigma)
  C. transport MLP 10->256->256->128 feature-major over cols (n,m,k)
     (vstar, 16384 cols) and (n,m) (v, 1024 cols), g = exp(tanh(.))
  D. scattering: U = sw @ g (w on partitions), DMA-transpose 128-col chunks
     to point layout, block-diag rw matmuls with the U-chunk as the
     stationary operand (so the result lands back in w layout with the
     output feature on partitions -> ACT bias works), tanh + residual;
     second round for the green function; quadrature over boundary points.
"""
import sys

if '/opt/trn_rl_repo' not in sys.path:
    sys.path.insert(0, '/opt/trn_rl_repo')

import numpy as np

import concourse.bass as bass
import concourse.bacc as bacc
import concourse.tile as tile
from concourse import mybir
from concourse.bass_utils import run_bass_kernel_spmd

F32 = mybir.dt.float32
BF16 = mybir.dt.bfloat16
AF = mybir.ActivationFunctionType
OP = mybir.AluOpType
AX = mybir.AxisListType

N_CORES = 8
B, N, M, K, P, C = 4, 32, 64, 16, 128, 2
NL = 16            # local n per core
K1 = K + 1         # 17
NK = NL * K1       # 272
HALF = NK // 2     # 136
PCOL = HALF * P    # 17408 attn cols per packed half
NC_VS = NL * M * K  # 16384 transport vstar cols, (n, m, k) k-inner
NC_V = NL * M       # 1024 transport v cols, (n, m)
ROWS3 = (128, 128, 16)   # NK partition tiles
ROW0 = (0, 128, 256)
CH = 2048


def ap_of(t, offset, pattern):
    return bass.AP(tensor=t, offset=offset, ap=pattern)


def rep_mid(src_ap, count):
    """Insert a stride-0 replication dim before the (contiguous) last dim."""
    a = list(src_ap.ap)
    return bass.AP(tensor=src_ap.tensor, offset=src_ap.offset,
                   ap=a[:-1] + [[0, count]] + a[-1:])


def build_program(debug=False):
    nc = bacc.Bacc("TRN2", target_bir_lowering=False, debug=False)
    dt = nc.dram_tensor

    T = {}

    def inp(name, shape, d=F32):
        T[name] = dt(name, shape, d, kind="ExternalInput")

    inp('xv', [NL, 4])
    inp('vallc', [NK, 2])
    inp('xbc', [NK, 2])
    inp('pcT', [2, P])
    inp('sgf', [P, C], BF16)
    inp('vwcol', [K, 1])
    inp('skT', [K, NL])
    inp('sskT', [K, K])
    inp('bndr', [1, M])
    inp('bwtr', [1, M])
    inp('aw0p', [12, 128], BF16)
    inp('ab0d', [128, 1])
    inp('aw1p', [128, 128], BF16)
    inp('ab1d', [128, 1])
    inp('aw2p', [128, 2], BF16)
    inp('qkxv', [8, PCOL], BF16)
    inp('tw0b', [10, 256], BF16)
    inp('tb0c', [128, 2])
    inp('tw1q', [128, 512], BF16)
    inp('tb1c', [128, 2])
    inp('tw2h', [128, 256], BF16)
    inp('tb2c', [128, 1])
    inp('trvs', [8, NC_VS], BF16)
    inp('trv', [8, NC_V], BF16)
    inp('sw0Tb', [128, 128], BF16)
    inp('sb0c', [128, 1])
    inp('sw1Tb', [128, 128], BF16)
    inp('sb1c', [128, 1])
    inp('selb', [128, 8], BF16)
    inp('ident', [128, 128])
    inp('outw', [128, 1])

    T['res'] = dt('res', [NL, 1], F32, kind="ExternalOutput")
    if debug:
        T['d_pl'] = dt('d_pl', [NK, P], F32, kind="ExternalOutput")
        T['d_att'] = dt('d_att', [NK, P], F32, kind="ExternalOutput")
        T['d_coef'] = dt('d_coef', [2, NK], F32, kind="ExternalOutput")
        T['d_gtv'] = dt('d_gtv', [128, NC_V], F32, kind="ExternalOutput")
        T['d_gtvs'] = dt('d_gtvs', [128, NC_VS], BF16, kind="ExternalOutput")
        T['d_log'] = dt('d_log', [2, PCOL], F32, kind="ExternalOutput")

    with tile.TileContext(nc) as tc:
        _emit(nc, tc, T, debug)
    nc.finalize()
    return nc


def _emit(nc, tc, T, debug):
    sync = nc.sync
    vec = nc.vector
    act = nc.scalar
    pe = nc.tensor

    with tc.tile_pool(name="singles", bufs=1) as singles, \
         tc.tile_pool(name="geo", bufs=1) as geo, \
         tc.tile_pool(name="big", bufs=1) as big, \
         tc.tile_pool(name="hch", bufs=2) as hch, \
         tc.tile_pool(name="dram", bufs=1, space="DRAM") as dram:
        rdi_scr = dram.tile([NL, P], F32)
        rwv_scr = dram.tile([K, NL], F32)
        cf_scr = dram.tile([2, NK], BF16)

        identt = singles.tile([128, 128], F32)
        sync.dma_start(out=identt[:], in_=T['ident'][:])
        epst = singles.tile([128, 1], F32)
        vec.memset(epst[:], 1e-16)

        # ================= Stage A: geometry =================
        xvt = geo.tile([NL, 4], F32)
        sync.dma_start(out=xvt[:], in_=T['xv'][:])
        VALL = [geo.tile([r, 2], F32, tag=f"vall{i}", name=f"vall{i}") for i, r in enumerate(ROWS3)]
        XB = [geo.tile([r, 2], F32, tag=f"xb{i}", name=f"xb{i}") for i, r in enumerate(ROWS3)]
        for i, r in enumerate(ROWS3):
            sync.dma_start(out=VALL[i][:], in_=T['vallc'][ROW0[i]:ROW0[i] + r, :])
            sync.dma_start(out=XB[i][:], in_=T['xbc'][ROW0[i]:ROW0[i] + r, :])

        ANG = [geo.tile([r, 2], F32, tag=f"ang{i}", name=f"ang{i}") for i, r in enumerate(ROWS3)]
        XDA = [geo.tile([r, 1], F32, tag=f"xda{i}", name=f"xda{i}") for i, r in enumerate(ROWS3)]
        for i, r in enumerate(ROWS3):
            sq = geo.tile([r, 2], F32, tag="geosq")
            vec.tensor_mul(sq[:], VALL[i][:], VALL[i][:])
            s1 = geo.tile([r, 1], F32, tag="geos1")
            vec.tensor_reduce(s1[:], sq[:], AX.X, OP.add)
            rn = geo.tile([r, 1], F32, tag="geonm")
            act.activation(out=rn[:], in_=s1[:], func=AF.Sqrt, bias=epst[:r, :], scale=1.0)
            rni = geo.tile([r, 1], F32, tag="georn")
            vec.reciprocal(rni[:], rn[:])
            vec.tensor_scalar_mul(ANG[i][:], VALL[i][:], rni[:])
            xa = geo.tile([r, 2], F32, tag="geoxa")
            vec.tensor_mul(xa[:], XB[i][:], ANG[i][:])
            vec.tensor_reduce(XDA[i][:], xa[:], AX.X, OP.add)

        PL = [geo.tile([r, P], F32, tag=f"pl{i}", name=f"pl{i}") for i, r in enumerate(ROWS3)]
        PLb = [geo.tile([r, P], BF16, tag=f"plb{i}", name=f"plb{i}") for i, r in enumerate(ROWS3)]
        PCT = geo.tile([2, P], F32)
        sync.dma_start(out=PCT[:], in_=T['pcT'][:])

        with tc.tile_pool(name="psgeo", bufs=1, space="PSUM") as psgeo:
            psT = psgeo.tile([2, NK], F32, tag="psang")
            for i, r in enumerate(ROWS3):
                pe.transpose(psT[:, ROW0[i]:ROW0[i] + r], ANG[i][:], identt[:r, :r])
            ANGT = geo.tile([2, NK], F32)
            vec.tensor_copy(out=ANGT[:], in_=psT[:])

            # pos_local = xdotang - ANG @ pc^T
            for i, r in enumerate(ROWS3):
                pd = psgeo.tile([128, P], F32, tag="pspd")
                pe.matmul(pd[:r, :], ANGT[:, ROW0[i]:ROW0[i] + r], PCT[:],
                          start=True, stop=True)
                act.activation(out=PL[i][:], in_=pd[:r, :], func=AF.Identity,
                               bias=XDA[i][:], scale=-1.0)
                vec.tensor_copy(out=PLb[i][:], in_=PL[i][:])

        # rel_dist reciprocal in n layout, then broadcast to NK via DRAM
        pcxb = geo.tile([NL, P], F32)
        pcyb = geo.tile([NL, P], F32)
        sync.dma_start(out=pcxb[:], in_=ap_of(T['pcT'], 0, [[0, NL], [1, P]]))
        sync.dma_start(out=pcyb[:], in_=ap_of(T['pcT'], P, [[0, NL], [1, P]]))
        relx = geo.tile([NL, P], F32)
        rely = geo.tile([NL, P], F32)
        vec.tensor_scalar(out=relx[:], in0=pcxb[:], scalar1=xvt[:, 0:1],
                          scalar2=None, op0=OP.subtract)
        vec.tensor_scalar(out=rely[:], in0=pcyb[:], scalar1=xvt[:, 1:2],
                          scalar2=None, op0=OP.subtract)
        r2 = geo.tile([NL, P], F32)
        vec.tensor_mul(r2[:], relx[:], relx[:])
        ry2 = geo.tile([NL, P], F32)
        vec.tensor_mul(ry2[:], rely[:], rely[:])
        vec.tensor_add(r2[:], r2[:], ry2[:])
        rd = geo.tile([NL, P], F32)
        act.activation(out=rd[:], in_=r2[:], func=AF.Sqrt, bias=epst[:NL, :], scale=1.0)
        vec.tensor_scalar_add(rd[:], rd[:], 1e-8)
        rdi = geo.tile([NL, P], F32)
        vec.reciprocal(rdi[:], rd[:])
        sync.dma_start(out=rdi_scr[:], in_=rdi[:])
        RDI = [geo.tile([r, P], F32, tag=f"rdi{i}", name=f"rdi{i}") for i, r in enumerate(ROWS3)]
        for n in range(NL):
            lo, hi = n * K1, (n + 1) * K1
            start = lo
            while start < hi:
                ti = 0 if start < 128 else (1 if start < 256 else 2)
                tlo = start - ROW0[ti]
                cnt = min(hi - start, ROWS3[ti] - tlo)
                sync.dma_start(out=RDI[ti][tlo:tlo + cnt, :],
                               in_=ap_of(rdi_scr.tensor, rdi_scr.offset + n * P, [[0, cnt], [1, P]]))
                start += cnt
        ALb = [geo.tile([r, P], BF16, tag=f"alb{i}", name=f"alb{i}") for i, r in enumerate(ROWS3)]
        for i in range(3):
            vec.tensor_mul(ALb[i][:], PL[i][:], RDI[i][:])

        # ================= Stage B: attention MLP (chunked) =================
        aw0t = singles.tile([12, 128], BF16)
        ab0t = singles.tile([128, 1], F32)
        aw1t = singles.tile([128, 128], BF16)
        ab1t = singles.tile([128, 1], F32)
        aw2t = singles.tile([128, 2], BF16)
        for t, s in ((aw0t, 'aw0p'), (ab0t, 'ab0d'), (aw1t, 'aw1p'),
                     (ab1t, 'ab1d'), (aw2t, 'aw2p')):
            sync.dma_start(out=t[:], in_=T[s][:])

        def nk_rows(row_start, nrows):
            done = 0
            while done < nrows:
                g = row_start + done
                ti = 0 if g < 128 else (1 if g < 256 else 2)
                tlo = g - ROW0[ti]
                cnt = min(nrows - done, ROWS3[ti] - tlo)
                yield ti, tlo, cnt, done
                done += cnt

        LG = [geo.tile([r, P], F32, tag=f"lg{i}", name=f"lg{i}")
              for i, r in enumerate(ROWS3)]
        with tc.tile_pool(name="psatt", bufs=2, space="PSUM") as psA, \
             tc.tile_pool(name="qkp", bufs=2) as qkp, \
             tc.tile_pool(name="logp", bufs=2) as logp:
            for c0 in range(0, PCOL, CH):
                cw = min(CH, PCOL - c0)
                kn0, knn = c0 // P, cw // P
                QKc = qkp.tile([12, CH], BF16, tag="qkc", name="qkc")
                sync.dma_start(out=QKc[0:4, :cw], in_=T['qkxv'][0:4, c0:c0 + cw])
                sync.dma_start(out=QKc[6:10, :cw], in_=T['qkxv'][4:8, c0:c0 + cw])
                for (row, srcs) in ((4, ALb), (5, PLb)):
                    for half, extra in ((0, 0), (1, 6)):
                        for ti, tlo, cnt, off in nk_rows(half * HALF + kn0, knn):
                            sync.dma_start(
                                out=QKc[row + extra:row + extra + 1,
                                        off * P:(off + cnt) * P]
                                .rearrange("a (b c) -> a b c", c=P),
                                in_=srcs[ti][tlo:tlo + cnt, :])
                t1 = psA.tile([128, CH], F32, tag="pa1", name="pa1")
                for s in range(0, cw, 512):
                    pe.matmul(t1[:, s:s + 512], aw0t[:], QKc[:, s:s + 512],
                              start=True, stop=True)
                h1c = hch.tile([128, CH], BF16, tag="h1c", name="h1c")
                act.activation(out=h1c[:, :cw], in_=t1[:, :cw], func=AF.Tanh, bias=ab0t[:])
                t2 = psA.tile([128, CH], F32, tag="pa1", name="pa2")
                for s in range(0, cw, 512):
                    pe.matmul(t2[:, s:s + 512], aw1t[:], h1c[:, s:s + 512],
                              start=True, stop=True)
                h2c = hch.tile([128, CH], BF16, tag="h2c", name="h2c")
                act.activation(out=h2c[:, :cw], in_=t2[:, :cw], func=AF.Tanh, bias=ab1t[:])
                t3 = psA.tile([128, CH], F32, tag="pa1", name="pa3")
                for s in range(0, cw, 512):
                    pe.matmul(t3[0:2, s:s + 512], aw2t[:], h2c[:, s:s + 512],
                              start=True, stop=True)
                # ab2 bias dropped: softmax is shift-invariant
                LOGc = logp.tile([2, CH], F32, tag="logc", name="logc")
                act.activation(out=LOGc[:, :cw], in_=t3[0:2, :cw],
                               func=AF.Identity, bias=0.0, scale=1.0)
                if debug:
                    sync.dma_start(out=T['d_log'][:, c0:c0 + cw], in_=LOGc[:, :cw])
                for half in (0, 1):
                    for ti, tlo, cnt, off in nk_rows(half * HALF + kn0, knn):
                        sync.dma_start(
                            out=LG[ti][tlo:tlo + cnt, :],
                            in_=LOGc[half:half + 1, off * P:(off + cnt) * P]
                            .rearrange("a (b c) -> a b c", c=P))

        # masked softmax over p
        negt = singles.tile([128, P], F32)
        vec.memset(negt[:], -1e30)
        ATT = [geo.tile([r, P], F32, tag=f"att{i}", name=f"att{i}") for i, r in enumerate(ROWS3)]
        for i, r in enumerate(ROWS3):
            mpos = geo.tile([r, P], mybir.dt.uint8, tag="mpos")
            vec.tensor_scalar(out=mpos[:], in0=PL[i][:], scalar1=0.0,
                              scalar2=None, op0=OP.is_gt)
            msk = geo.tile([r, P], F32, tag="msk")
            vec.select(msk[:], mpos[:], LG[i][:], negt[:r, :])
            nmx = geo.tile([r, 1], F32, tag="nmx")
            vec.tensor_reduce(nmx[:], msk[:], AX.X, OP.max, negate=True)
            ssum = geo.tile([r, 1], F32, tag="ssum")
            act.activation(out=ATT[i][:], in_=msk[:], func=AF.Exp,
                           bias=nmx[:], scale=1.0, accum_out=ssum[:])
            rs = geo.tile([r, 1], F32, tag="rs")
            vec.reciprocal(rs[:], ssum[:])
            vec.tensor_scalar_mul(ATT[i][:], ATT[i][:], rs[:])
        if debug:
            for i, r in enumerate(ROWS3):
                sync.dma_start(out=T['d_att'][ROW0[i]:ROW0[i] + r, :], in_=ATT[i][:])
                sync.dma_start(out=T['d_pl'][ROW0[i]:ROW0[i] + r, :], in_=PL[i][:])

        # coeff^T = exp(-sigma^T @ ATT^T)  [2, 272]
        cfb = geo.tile([2, NK], BF16)
        with tc.tile_pool(name="pscf", bufs=1, space="PSUM") as pscf:
            psAT = pscf.tile([128, NK], F32, tag="psat")
            for i, r in enumerate(ROWS3):
                pe.transpose(psAT[:, ROW0[i]:ROW0[i] + r], ATT[i][:], identt[:r, :r])
            ATTTb = geo.tile([128, NK], BF16)
            vec.tensor_copy(out=ATTTb[:], in_=psAT[:])
            sgt = singles.tile([P, C], BF16)
            sync.dma_start(out=sgt[:], in_=T['sgf'][:])
            psZ = pscf.tile([2, NK], F32, tag="psz")
            pe.matmul(psZ[:], sgt[:], ATTTb[:], start=True, stop=True)
            COEF = geo.tile([2, NK], F32)
            act.activation(out=COEF[:], in_=psZ[:], func=AF.Exp, bias=0.0, scale=-1.0)
            vec.tensor_copy(out=cfb[:], in_=COEF[:])
            if debug:
                sync.dma_start(out=T['d_coef'][:], in_=COEF[:])

        # ================= Stage C: transport MLP =================
        rv = big.tile([10, NC_V], BF16)
        sync.dma_start(out=rv[0:8, :], in_=T['trv'][:])
        # rv coeff rows (k1=0 values) via DRAM bounce + partition-bcast
        sync.dma_start(out=cf_scr[:], in_=cfb[:])
        for d in range(2):
            for n in range(NL):
                cmv = geo.tile([M, 1], BF16, tag="cmv", name="cmv")
                sync.dma_start(out=cmv[:],
                               in_=ap_of(cf_scr.tensor, cf_scr.offset + d * NK + n * K1,
                                         [[0, M], [1, 1]]))
                sync.dma_start(out=rv[8 + d:9 + d, n * M:(n + 1) * M], in_=cmv[:])

        tw0t = singles.tile([10, 256], BF16)
        tw1t = singles.tile([128, 512], BF16)
        tw2t = singles.tile([128, 256], BF16)
        tb0t = singles.tile([128, 2], F32)
        tb1t = singles.tile([128, 2], F32)
        tb2t = singles.tile([128, 1], F32)
        for t, s in ((tw0t, 'tw0b'), (tw1t, 'tw1q'), (tw2t, 'tw2h'),
                     (tb0t, 'tb0c'), (tb1t, 'tb1c'), (tb2t, 'tb2c')):
            sync.dma_start(out=t[:], in_=T[s][:])

        GTvs = big.tile([128, NC_VS], BF16)
        GTv = big.tile([128, NC_V], F32)
        CHT = 1024

        def transport(rhs, c0g, ncols, gt_store, psA):
            for c0 in range(0, ncols, CHT):
                cw = min(CHT, ncols - c0)
                ta = psA.tile([128, CHT], F32, tag="tta", name="tta")
                tb = psA.tile([128, CHT], F32, tag="ttb", name="ttb")
                for s in range(0, cw, 512):
                    pe.matmul(ta[:, s:s + 512], tw0t[:, 0:128],
                              rhs[:, c0 + s:c0 + s + 512], start=True, stop=True)
                    pe.matmul(tb[:, s:s + 512], tw0t[:, 128:256],
                              rhs[:, c0 + s:c0 + s + 512], start=True, stop=True)
                h1a = hch.tile([128, CHT], BF16, tag="th1a", name="th1a")
                h1b = hch.tile([128, CHT], BF16, tag="th1b", name="th1b")
                act.activation(out=h1a[:, :cw], in_=ta[:, :cw], func=AF.Tanh,
                               bias=tb0t[:, 0:1])
                act.activation(out=h1b[:, :cw], in_=tb[:, :cw], func=AF.Tanh,
                               bias=tb0t[:, 1:2])
                t2a = psA.tile([128, CHT], F32, tag="tta", name="t2a")
                t2b = psA.tile([128, CHT], F32, tag="ttb", name="t2b")
                for s in range(0, cw, 512):
                    pe.matmul(t2a[:, s:s + 512], tw1t[:, 0:128], h1a[:, s:s + 512],
                              start=True, stop=False)
                    pe.matmul(t2a[:, s:s + 512], tw1t[:, 256:384], h1b[:, s:s + 512],
                              start=False, stop=True)
                    pe.matmul(t2b[:, s:s + 512], tw1t[:, 128:256], h1a[:, s:s + 512],
                              start=True, stop=False)
                    pe.matmul(t2b[:, s:s + 512], tw1t[:, 384:512], h1b[:, s:s + 512],
                              start=False, stop=True)
                h2a = hch.tile([128, CHT], BF16, tag="th1a", name="h2a")
                h2b = hch.tile([128, CHT], BF16, tag="th1b", name="h2b")
                act.activation(out=h2a[:, :cw], in_=t2a[:, :cw], func=AF.Tanh,
                               bias=tb1t[:, 0:1])
                act.activation(out=h2b[:, :cw], in_=t2b[:, :cw], func=AF.Tanh,
                               bias=tb1t[:, 1:2])
                te = psA.tile([128, CHT], F32, tag="tta", name="te")
                for s in range(0, cw, 512):
                    pe.matmul(te[:, s:s + 512], tw2t[:, 0:128], h2a[:, s:s + 512],
                              start=True, stop=False)
                    pe.matmul(te[:, s:s + 512], tw2t[:, 128:256], h2b[:, s:s + 512],
                              start=False, stop=True)
                tnh = hch.tile([128, CHT], BF16, tag="ttnh", name="tnh")
                act.activation(out=tnh[:, :cw], in_=te[:, :cw], func=AF.Tanh,
                               bias=tb2t[:])
                act.activation(out=gt_store[:, c0g + c0:c0g + c0 + cw],
                               in_=tnh[:, :cw], func=AF.Exp, bias=0.0, scale=1.0)

        with tc.tile_pool(name="pstr", bufs=2, space="PSUM") as psT2, \
             tc.tile_pool(name="rvsp", bufs=1) as rvsp:
            for hf in range(2):
                rvsh = rvsp.tile([10, NC_VS // 2], BF16, tag="rvsh", name="rvsh")
                sync.dma_start(out=rvsh[0:8, :],
                               in_=T['trvs'][0:8, hf * 8192:(hf + 1) * 8192])
                for d in range(2):
                    for n in range(8):
                        gn = hf * 8 + n
                        src = cfb[d:d + 1, gn * K1 + 1:gn * K1 + 1 + K]
                        sync.dma_start(
                            out=rvsh[8 + d:9 + d, n * M * K:(n + 1) * M * K]
                            .rearrange("a (b c) -> a b c", c=K),
                            in_=rep_mid(src, M))
                transport(rvsh, hf * 8192, 8192, GTvs, psT2)
            transport(rv, 0, NC_V, GTv, psT2)

        # ================= Stage D: scattering =================
        sskTt = geo.tile([K, K], F32)
        skTt = geo.tile([K, NL], F32)
        vwt = geo.tile([K, 1], F32)
        sync.dma_start(out=sskTt[:], in_=T['sskT'][:])
        sync.dma_start(out=skTt[:], in_=T['skT'][:])
        sync.dma_start(out=vwt[:], in_=T['vwcol'][:])
        rwvsT = geo.tile([K, K], F32)
        vec.tensor_scalar(out=rwvsT[:], in0=sskTt[:], scalar1=-1.0, scalar2=1.0,
                          op0=OP.mult, op1=OP.add)
        vec.tensor_scalar_mul(rwvsT[:], rwvsT[:], vwt[:])
        rwvT = geo.tile([K, NL], F32)
        vec.tensor_scalar(out=rwvT[:], in0=skTt[:], scalar1=-1.0, scalar2=1.0,
                          op0=OP.mult, op1=OP.add)
        vec.tensor_scalar_mul(rwvT[:], rwvT[:], vwt[:])
        rwvsTb = geo.tile([K, K], BF16)
        vec.tensor_copy(out=rwvsTb[:], in_=rwvsT[:])
        rwblk = singles.tile([128, 128], BF16)
        vec.memset(rwblk[:], 0.0)
        for p8 in range(8):
            sync.dma_start(out=rwblk[p8 * K:(p8 + 1) * K, p8 * K:(p8 + 1) * K],
                           in_=rwvsTb[:])
        sync.dma_start(out=rwv_scr[:], in_=rwvT[:])
        RWVS = geo.tile([128, NL], F32)
        for p8 in range(8):
            sync.dma_start(out=RWVS[p8 * K:(p8 + 1) * K, :], in_=rwv_scr[:])
        selt = singles.tile([128, 8], BF16)
        sync.dma_start(out=selt[:], in_=T['selb'][:])
        RWVC = singles.tile([128, 128], BF16)
        for n in range(NL):
            vec.tensor_scalar_mul(RWVC[:, n * 8:(n + 1) * 8], selt[:], RWVS[:, n:n + 1])

        sw0t = singles.tile([128, 128], BF16)
        sw1t = singles.tile([128, 128], BF16)
        sb0t = singles.tile([128, 1], F32)
        sb1t = singles.tile([128, 1], F32)
        for t, s in ((sw0t, 'sw0Tb'), (sw1t, 'sw1Tb'), (sb0t, 'sb0c'), (sb1t, 'sb1c')):
            sync.dma_start(out=t[:], in_=T[s][:])

        with tc.tile_pool(name="psu", bufs=2, space="PSUM") as psU, \
             tc.tile_pool(name="psvs", bufs=1, space="PSUM") as psVS, \
             tc.tile_pool(name="psv", bufs=2, space="PSUM") as psV, \
             tc.tile_pool(name="usb", bufs=2) as usb, \
             tc.tile_pool(name="upt", bufs=4) as upt, \
             tc.tile_pool(name="t0p", bufs=2) as t0p:
            for rnd in range(2):
                swt = sw0t if rnd == 0 else sw1t
                sbt = sb0t if rnd == 0 else sb1t
                pv = None
                for g in range(16):
                    pu = psU.tile([128, 1024], F32, tag="pu")
                    for s in range(0, 1024, 512):
                        pe.matmul(pu[:, s:s + 512], swt[:],
                                  GTvs[:, g * 1024 + s:g * 1024 + s + 512],
                                  start=True, stop=True)
                    ub = usb.tile([128, 1024], BF16, tag="ub")
                    vec.tensor_copy(out=ub[:], in_=pu[:])
                    if rnd == 0:
                        pvs = psVS.tile([128, 1024], F32, tag="pvs")
                    if g % 8 == 0:
                        pv = psV.tile([128, 512], F32, tag="pv")
                    for j in range(8):
                        t = g * 8 + j
                        up = upt.tile([128, 128], BF16, tag="up")
                        sync.dma_start_transpose(up[:], ub[:, j * 128:(j + 1) * 128])
                        n = t // 8
                        if rnd == 0:
                            pe.matmul(pvs[:, j * 128:(j + 1) * 128], up[:], rwblk[:],
                                      start=True, stop=True)
                        pe.matmul(pv[:, (t % 64) * 8:(t % 64) * 8 + 8], up[:],
                                  RWVC[:, n * 8:(n + 1) * 8], start=True, stop=True)
                    if rnd == 0:
                        t0c = t0p.tile([128, 1024], BF16, tag="t0c")
                        act.activation(out=t0c[:], in_=pvs[:], func=AF.Tanh, bias=sbt[:])
                        vec.tensor_add(GTvs[:, g * 1024:(g + 1) * 1024],
                                       GTvs[:, g * 1024:(g + 1) * 1024], t0c[:])
                    if g % 8 == 7:
                        tv = t0p.tile([128, 512], F32, tag="tv")
                        act.activation(out=tv[:], in_=pv[:], func=AF.Tanh, bias=sbt[:])
                        half = g // 8
                        vec.tensor_add(GTv[:, half * 512:(half + 1) * 512],
                                       GTv[:, half * 512:(half + 1) * 512], tv[:])
        if debug:
            sync.dma_start(out=T['d_gtvs'][:], in_=GTvs[:])
            sync.dma_start(out=T['d_gtv'][:], in_=GTv[:])

        # quadrature
        outwt = singles.tile([128, 1], F32)
        sync.dma_start(out=outwt[:], in_=T['outw'][:])
        with tc.tile_pool(name="psg", bufs=1, space="PSUM") as psgp:
            psG = psgp.tile([1, NC_V], F32, tag="psg")
            for s in range(0, NC_V, 512):
                pe.matmul(psG[:, s:s + 512], outwt[:], GTv[:, s:s + 512],
                          start=True, stop=True)
            gdr = geo.tile([1, NC_V], F32)
            act.activation(out=gdr[:], in_=psG[:], func=AF.Identity, bias=0.0, scale=1.0)
        GD = geo.tile([NL, M], F32)
        sync.dma_start(out=GD[:], in_=gdr[:].rearrange("a (b c) -> a b c", c=M))
        bnd16 = geo.tile([NL, M], F32)
        bwt16 = geo.tile([NL, M], F32)
        sync.dma_start(out=bnd16[:], in_=ap_of(T['bndr'], 0, [[0, NL], [1, M]]))
        sync.dma_start(out=bwt16[:], in_=ap_of(T['bwtr'], 0, [[0, NL], [1, M]]))
        vec.tensor_mul(bnd16[:], bnd16[:], bwt16[:])
        vec.tensor_mul(GD[:], GD[:], bnd16[:])
        rest = geo.tile([NL, 1], F32)
        vec.tensor_reduce(rest[:], GD[:], AX.X, OP.add)
        sync.dma_start(out=T['res'][:], in_=rest[:])


_CACHE = {}


def _get_program(debug=False):
    key = ('nc', debug)
    if key not in _CACHE:
        _CACHE[key] = build_program(debug)
    return _CACHE[key]


def _to_bf16(a):
    import ml_dtypes
    return np.asarray(a, np.float32).astype(ml_dtypes.bfloat16)


def host_prep(inputs, c):
    b, n0 = c // 2, (c % 2) * NL
    f32 = lambda a: np.ascontiguousarray(np.asarray(a), dtype=np.float32)

    xv = f32(inputs['phase_coords'][b, n0:n0 + NL])
    vc = f32(inputs['velocity_coords'][b])
    x, v = xv[:, :2], xv[:, 2:]
    vall = np.zeros((NK, 2), np.float32)
    xb = np.zeros((NK, 2), np.float32)
    for n in range(NL):
        vall[n * K1] = v[n]
        vall[n * K1 + 1:(n + 1) * K1] = vc
        xb[n * K1:(n + 1) * K1] = x[n]
    pc = f32(inputs['position_coords'][b])
    bc = f32(inputs['boundary_coords'][b])

    qkxv = np.zeros((8, PCOL), np.float32)
    for h, (lo, hi) in enumerate(((0, HALF), (HALF, NK))):
        qkxv[4 * h + 0] = np.repeat(xb[lo:hi, 0], P)
        qkxv[4 * h + 1] = np.repeat(xb[lo:hi, 1], P)
        qkxv[4 * h + 2] = np.repeat(vall[lo:hi, 0], P)
        qkxv[4 * h + 3] = np.repeat(vall[lo:hi, 1], P)

    trvs = np.zeros((8, NC_VS), np.float32)
    trv = np.zeros((8, NC_V), np.float32)
    for d in range(2):
        trvs[d] = np.repeat(x[:, d], M * K)
        trv[d] = np.repeat(x[:, d], M)
        trvs[2 + d] = np.tile(vc[:, d], NL * M)
        trv[2 + d] = np.repeat(v[:, d], M)
    for d in range(4):
        trvs[4 + d] = np.tile(np.repeat(bc[:, d], K), NL)
        trv[4 + d] = np.tile(bc[:, d], NL)

    aw0, ab0 = f32(inputs['aw0']), f32(inputs['ab0'])
    aw1, ab1 = f32(inputs['aw1']), f32(inputs['ab1'])
    aw2 = f32(inputs['aw2'])
    aw0p = np.zeros((12, 128), np.float32)
    aw0p[0:6, 0:64] = aw0
    aw0p[6:12, 64:128] = aw0
    aw1p = np.zeros((128, 128), np.float32)
    aw1p[0:64, 0:64] = aw1
    aw1p[64:128, 64:128] = aw1
    aw2p = np.zeros((128, 2), np.float32)
    aw2p[0:64, 0] = aw2[:, 0]
    aw2p[64:128, 1] = aw2[:, 0]

    tw0, tb0 = f32(inputs['tw0']), f32(inputs['tb0'])
    tw1, tb1 = f32(inputs['tw1']), f32(inputs['tb1'])
    tw2, tb2 = f32(inputs['tw2']), f32(inputs['tb2'])
    tw1q = np.zeros((128, 512), np.float32)
    for kh in range(2):
        for mh in range(2):
            tw1q[:, (kh * 2 + mh) * 128:(kh * 2 + mh + 1) * 128] = \
                tw1[kh * 128:(kh + 1) * 128, mh * 128:(mh + 1) * 128]
    tw2h = np.concatenate([tw2[0:128, :], tw2[128:256, :]], axis=1)

    sel = np.zeros((128, 8), np.float32)
    for p8 in range(8):
        sel[p8 * K:(p8 + 1) * K, p8] = 1.0

    bf = _to_bf16
    return {
        'xv': xv, 'vallc': vall, 'xbc': xb,
        'pcT': np.ascontiguousarray(pc.T),
        'sgf': bf(inputs['sigma'][b]),
        'vwcol': f32(inputs['velocity_weights'][b])[:, None],
        'skT': np.ascontiguousarray(f32(inputs['scattering_kernel'][b, n0:n0 + NL]).T),
        'sskT': np.ascontiguousarray(f32(inputs['self_scattering_kernel'][b]).T),
        'bndr': f32(inputs['boundary'][b])[None, :],
        'bwtr': f32(inputs['boundary_weights'][b])[None, :],
        'aw0p': bf(aw0p),
        'ab0d': np.concatenate([ab0, ab0])[:, None].astype(np.float32),
        'aw1p': bf(aw1p),
        'ab1d': np.concatenate([ab1, ab1])[:, None].astype(np.float32),
        'aw2p': bf(aw2p),
        'qkxv': bf(qkxv),
        'tw0b': bf(tw0),
        'tb0c': np.ascontiguousarray(tb0.reshape(2, 128).T),
        'tw1q': bf(tw1q),
        'tb1c': np.ascontiguousarray(tb1.reshape(2, 128).T),
        'tw2h': bf(tw2h),
        'tb2c': tb2[:, None].astype(np.float32),
        'trvs': bf(trvs), 'trv': bf(trv),
        'sw0Tb': bf(np.ascontiguousarray(f32(inputs['sw0']).T)),
        'sb0c': f32(inputs['sb0'])[:, None],
        'sw1Tb': bf(np.ascontiguousarray(f32(inputs['sw1']).T)),
        'sb1c': f32(inputs['sb1'])[:, None],
        'selb': bf(sel),
        'ident': np.eye(128, dtype=np.float32),
        'outw': f32(inputs['out_w'])[:, None],
    }


def kernel(**inputs):
    nc = _get_program()
    in_maps = [host_prep(inputs, c) for c in range(N_CORES)]
    r = run_bass_kernel_spmd(nc, in_maps, list(range(N_CORES)))
    out = np.zeros((B, N), np.float32)
    for c in range(N_CORES):
        b, n0 = c // 2, (c % 2) * NL
        out[b, n0:n0 + NL] = r.results[c]['res'][:, 0]
    return out
